# revision 11
# baseline (speedup 1.0000x reference)
"""Trainium2 Bass kernel for DeformableConvBlock (B=4, C=64, H=W=128, K=3).

Self-contained: builds an SPMD Bass/Tile program for 8 NeuronCores.
Core c handles image c//2, output-row half c%2 (data-parallel over
batch x row-halves).

v3 pipeline. Key constraint discovered on HW: the GpSimd Q7 SWDGE
descriptor generation (dma_gather) shares an SBUF port pair with the
DVE's 2-port perf modes, so any tensor_scalar/tensor_copy/memset issued
while a gather is generating blocks until that generation finishes
(~36us). The steady-state therefore uses ONLY contention-free DVE ops
(tensor_tensor / scalar_tensor_tensor / stream_shuffle); index
replication runs as SBUF->SBUF HWDGE DMAs, and the s8 pad columns live
in two persistent buffers zeroed once up front.

Front end per row-section ([8,8,16,16,16]): offset-conv (PE) ->
DMA-transpose to pixel-major -> fused bilinear weight/index prep (DVE)
-> wrapped-index stream_shuffles + replicate DMAs -> dma_gathers
(4608 idx x 512B each, queues round-robin; 4 queues generate
concurrently at ~7.9ns/idx -> 146us floor for 16 gathers).
Bilinear consumes (DVE mult + 2 adds), transpose, PE contraction, bias,
out-DMA are interleaved a few sections behind the gather stream.

kernel(**inputs) takes the full unsharded numpy inputs and returns the
full [4, 64, 128, 128] float32 output.
"""
from contextlib import ExitStack

import numpy as np
import ml_dtypes

import concourse.bacc as bacc
import concourse.bass as bass
import concourse.mybir as mybir
import concourse.tile as tile
from concourse.tile import TileContext
from concourse.vector_clock import ScopedClock, VectorClock

F32 = mybir.dt.float32
BF16 = mybir.dt.bfloat16
I32 = mybir.dt.int32
I16 = mybir.dt.int16
AF = mybir.ActivationFunctionType
OP = mybir.AluOpType

H = W = 128
C = 64
O = 64
KK = 9
ROWS = 64            # output rows per core
PADT = 2             # table padding on each side
PW = W + 2 * PADT    # 132
NTAB = PW * PW       # 17424 table rows
EROW = 4 * C         # 256 elems per table row
KTILES = 9           # 1152 = 9*128 contraction tiles: (k, c, x-corner pair)
GROUP = 4            # rows per gather group = rows per matmul batch
XBW = W + 3          # xband2 column count
BIG = 12582912.0     # 1.5 * 2^23 fp32 rounding offset
SECS = [4, 12, 16, 16, 16]


class TileContextSplitDrain(TileContext):
    """Stock epilogue emits one Drain with one wait per outstanding proc;
    this walrus rejects >1 sync wait per instruction, so emit one Drain
    per proc instead."""

    def _drain_and_barrier(self, tick_clock, wait_clock):
        gc = tick_clock.global_clock
        nprocs = len(gc)
        emitted = False
        for p in range(nprocs):
            t = gc[p]
            if t <= 0:
                continue
            vec = [0] * nprocs
            vec[p] = t
            drain_inst = self.nc.sync.drain()
            wait_clock.add_sem_waits(
                drain_inst.ins, ScopedClock({None: VectorClock(vec)})
            )
            si = drain_inst.ins.sync_info
            assert si is None or len(si.on_wait) <= 1
            emitted = True
        if not emitted:
            self.nc.sync.drain()
        self.nc.all_engine_barrier()
        assert self.sems is not None
        popped = self.nc._tile_sem_poison_stack.pop()
        assert popped is self._sem_poison
        self.nc.clear_and_free_semaphores(list(self.sems.allocated().values()))
        self.nc.all_engine_barrier()


def build_program(nrows=ROWS, g_bufs=6):
    npx = nrows * W
    secs = SECS if nrows >= 64 else [nrows]

    nc = bacc.Bacc(num_swdge_queues=4)
    # xband2: rows 0-63 channel c of padded x (cols 0..XBW-1), rows 64-127
    # the same shifted one column left (cols 1..XBW).
    xband2 = nc.dram_tensor("xband2", [128, nrows + 2, XBW], BF16, kind="ExternalInput")
    table = nc.dram_tensor("table", [NTAB, EROW], BF16, kind="ExternalInput")
    w_off = nc.dram_tensor("w_off", [KK, C, 18], BF16, kind="ExternalInput")
    w_offp = nc.dram_tensor("w_offp", [3, 128, 18], BF16, kind="ExternalInput")
    b_off = nc.dram_tensor("b_off", [18, 1], F32, kind="ExternalInput")
    w2 = nc.dram_tensor("w2", [KTILES, 128, O], BF16, kind="ExternalInput")
    b2 = nc.dram_tensor("b2", [O, 1], F32, kind="ExternalInput")
    cgridy = nc.dram_tensor("cgridy", [1, nrows * KK], BF16, kind="ExternalInput")
    cxfull = nc.dram_tensor("cxfull", [128, nrows * KK], BF16, kind="ExternalInput")
    cvals = nc.dram_tensor("cvals", [1, 4], F32, kind="ExternalInput")
    cone = nc.dram_tensor("cone", [1, 1], BF16, kind="ExternalInput")
    out = nc.dram_tensor("out", [O, npx], F32, kind="ExternalOutput")

    ctx = ExitStack()
    with TileContextSplitDrain(nc) as tc:
        # wrapped-idx tiles only fill the 32-partition block the gather's
        # queue actually reads; other blocks keep stale (in-bounds) data
        # that the race detector would flag
        tc.race_detector_enabled = False
        const_pool = ctx.enter_context(tc.tile_pool(name="const", bufs=1))
        big_pool = ctx.enter_context(tc.tile_pool(name="big", bufs=1))
        prep_pool = ctx.enter_context(tc.tile_pool(name="prep", bufs=1))
        oc_pool = ctx.enter_context(tc.tile_pool(name="oc", bufs=2))
        wr_pool = ctx.enter_context(tc.tile_pool(name="wr", bufs=2))
        g_pool = ctx.enter_context(tc.tile_pool(name="g", bufs=g_bufs))
        st_pool = ctx.enter_context(tc.tile_pool(name="st", bufs=2))
        o_pool = ctx.enter_context(tc.tile_pool(name="o", bufs=2))
        psum_pool = ctx.enter_context(tc.tile_pool(name="ps", bufs=2, space="PSUM"))
        psum2_pool = ctx.enter_context(tc.tile_pool(name="ps2", bufs=2, space="PSUM"))

        # ---- constant loads ----
        xb = const_pool.tile([128, (nrows + 2) * XBW], BF16)
        xb_v = xb[:].rearrange("c (h w) -> c h w", h=nrows + 2, w=XBW)
        # split band load so the first offset-conv section starts early
        xsplit = min(secs[0] + 6, nrows + 2)
        nc.sync.dma_start(out=xb_v[:, 0:xsplit], in_=xband2[:, 0:xsplit])
        nc.sync.dma_start(out=xb_v[:, xsplit:], in_=xband2[:, xsplit:])

        wof = const_pool.tile([C, KK * 18], BF16)
        wof_v = wof[:].rearrange("c (k e) -> c k e", k=KK, e=18)
        nc.sync.dma_start(out=wof_v, in_=w_off[:].rearrange("k c e -> c k e"))

        wofp = const_pool.tile([128, 3 * 18], BF16)
        wofp_v = wofp[:].rearrange("c (k e) -> c k e", k=3, e=18)
        nc.sync.dma_start(out=wofp_v, in_=w_offp[:].rearrange("k c e -> c k e"))

        bof = const_pool.tile([18, 1], F32)
        nc.sync.dma_start(out=bof[:], in_=b_off[:])

        w2t = const_pool.tile([128, KTILES * O], BF16)
        w2t_v = w2t[:].rearrange("p (j e) -> p j e", j=KTILES, e=O)
        nc.sync.dma_start(out=w2t_v, in_=w2[:].rearrange("j p e -> p j e"))

        b2t = const_pool.tile([O, 1], F32)
        nc.sync.dma_start(out=b2t[:], in_=b2[:])

        cy = const_pool.tile([128, nrows * KK], BF16)
        nc.sync.dma_start(out=cy[:], in_=cgridy[0:1, :].to_broadcast((128, nrows * KK)))
        cxf = const_pool.tile([128, nrows * KK], BF16)
        nc.sync.dma_start(out=cxf[:], in_=cxfull[:])
        cv = const_pool.tile([128, 4], F32)
        nc.sync.dma_start(out=cv[:], in_=cvals[0:1, :].to_broadcast((128, 4)))
        one_b = const_pool.tile([128, 1], BF16)
        nc.sync.dma_start(out=one_b[:], in_=cone[0:1, :].to_broadcast((128, 1)))

        def cvb(i, n):  # broadcast view of constant column i
            return cv[:, i:i + 1].to_broadcast((128, n))

        # ---- persistent whole-half tiles ----
        op = big_pool.tile([128, nrows * 32], BF16)
        op3 = op[:].rearrange("p (y e) -> p y e", y=nrows, e=32)
        wf = big_pool.tile([128, nrows * 36], BF16)
        wf_v = wf[:].rearrange("p (y k c) -> p y k c", y=nrows, k=KK, c=4)
        idx16 = big_pool.tile([128, nrows * KK], I16)
        # two persistent s8 buffers (1152 = 9*64*2 used exactly, no padding)
        s8ab = [
            big_pool.tile([128, GROUP * KTILES * 128], BF16,
                          tag=f"s8_{sbi}", name=f"s8_{sbi}")
            for sbi in range(2)
        ]

        ptiles = {}

        def pp(tag, dt=F32):
            if tag not in ptiles:
                ptiles[tag] = prep_pool.tile(
                    [128, nrows * KK], dt, tag=tag, name=f"prep_{tag}")
            return ptiles[tag]

        gtiles = [None] * (nrows // GROUP)

        def emit_consume(gi):
            y0 = gi * GROUP
            g = gtiles[gi]
            g_v = g[:].rearrange(
                "p (y k c e) -> p y k c e", y=GROUP, k=KK, c=C, e=4)
            wfb = wf_v[:, y0:y0 + GROUP, :, None, :].to_broadcast(
                (128, GROUP, KK, C, 4))
            nc.vector.tensor_tensor(out=g_v, in0=g_v, in1=wfb, op=OP.mult)
            # y-corner pairwise add, compacted into s8: s8[y,k,c,xe] =
            # g[...,xe] + g[...,xe+2]; the x-corner pair stays in the PE
            # contraction (weights duplicated per pair)
            s8 = s8ab[gi % 2]
            s8_v = s8[:].rearrange(
                "p (y k c e) -> p y k c e", y=GROUP, k=KK, c=C, e=2)
            nc.vector.tensor_tensor(
                out=s8_v, in0=g_v[:, :, :, :, 0:2],
                in1=g_v[:, :, :, :, 2:4], op=OP.add)

            st = st_pool.tile([128, GROUP * KTILES * 128], BF16, tag="st")
            st_v = st[:].rearrange(
                "p (m c) -> p m c", m=GROUP * KTILES, c=128)
            nc.sync.dma_start_transpose(out=st_v, in_=s8[:])
            st_y = st[:].rearrange(
                "p (y j c) -> p y j c", y=GROUP, j=KTILES, c=128)
            ps2 = psum2_pool.tile([O, 512], F32, tag="ps2")
            for j in range(KTILES):
                nc.tensor.matmul(
                    out=ps2[:], lhsT=w2t_v[:, j, :], rhs=st_y[:, :, j, :],
                    start=(j == 0), stop=(j == KTILES - 1),
                )
            ob = o_pool.tile([O, 512], F32, tag="ob")
            nc.scalar.activation(
                out=ob[:], in_=ps2[:], func=AF.Identity, bias=b2t[:])
            nc.sync.dma_start(
                out=out[:, gi * 512:(gi + 1) * 512], in_=ob[:])

        # ---------------- sections ----------------
        row0 = 0
        gi0 = 0
        for si, srows in enumerate(secs):
            ysl = slice(row0, row0 + srows)
            msl = slice(row0 * KK, (row0 + srows) * KK)
            mlen = srows * KK

            # offset conv: per 4-row chunk, 3 tap-pair matmuls
            # (contraction 128 = taps (dy,0)+(dy,1)) + 3 singles (dy,2)
            oct_ = oc_pool.tile([32, max(secs) * W], BF16, tag="ocq")
            ocq = oct_[:, 0:srows * W]
            if si < 2:  # oc buffers only ever written on rows 0:18; zero once
                nc.scalar.memzero(oct_[:])
            for cc in range(srows // 4):
                r0 = row0 + cc * 4
                ps = psum_pool.tile([18, 512], F32, tag="ps1")
                for dy in range(3):
                    nc.tensor.matmul(
                        out=ps[:], lhsT=wofp_v[:, dy, :],
                        rhs=xb_v[:, r0 + dy:r0 + dy + 4, 0:W],
                        start=(dy == 0), stop=False,
                    )
                for dy in range(3):
                    nc.tensor.matmul(
                        out=ps[:], lhsT=wof_v[:, 3 * dy + 2, :],
                        rhs=xb_v[0:C, r0 + dy:r0 + dy + 4, 2:2 + W],
                        start=False, stop=(dy == 2),
                    )
                nc.scalar.activation(
                    out=ocq[:18, cc * 512:(cc + 1) * 512], in_=ps[:],
                    func=AF.Identity, bias=bof[:],
                )

            # offsets to pixel-major
            op3q = op3[:, ysl, :]
            nc.sync.dma_start_transpose(out=op3q, in_=ocq)

            # ---- wf/idx prep: contention-free ops only (TT / STT) ----
            s = pp("s")
            f0 = pp("f0")
            lt = pp("lt")
            s3 = s[:].rearrange("p (y k) -> p y k", y=nrows, k=KK)
            for axis in (0, 1):  # 0: y, 1: x
                grid = cy if axis == 0 else cxf
                grid3 = grid[:].rearrange("p (y k) -> p y k", y=nrows, k=KK)
                nc.vector.tensor_tensor(
                    out=s3[:, ysl], in0=op3q[:, :, axis * 9:axis * 9 + 9],
                    in1=grid3[:, ysl], op=OP.add)
                # round to nearest via +-1.5*2^23 (stage0 scalar add rounds,
                # stage1 tensor subtract undoes; no constant folding across
                # ALU stages), then floor = round - (s < round)
                nc.vector.scalar_tensor_tensor(
                    out=f0[:, msl], in0=s[:, msl], scalar=BIG,
                    in1=cvb(0, mlen), op0=OP.add, op1=OP.subtract)
                nc.vector.tensor_tensor(
                    out=lt[:, msl], in0=s[:, msl], in1=f0[:, msl], op=OP.is_lt)
                nc.vector.tensor_tensor(
                    out=f0[:, msl], in0=f0[:, msl], in1=lt[:, msl],
                    op=OP.subtract)
                # fractions
                w1 = pp(f"w1_{axis}", BF16)
                nc.vector.tensor_tensor(
                    out=w1[:, msl], in0=s[:, msl], in1=f0[:, msl],
                    op=OP.subtract)
                w0 = pp(f"w0_{axis}", BF16)
                nc.vector.scalar_tensor_tensor(
                    out=w0[:, msl], in0=w1[:, msl], scalar=-1.0,
                    in1=one_b[:].to_broadcast((128, mlen)),
                    op0=OP.mult, op1=OP.add)
                # clamp for index (clamped coords land in the zero pad);
                # x additionally folds in the +266 table offset
                c0 = pp(f"c0_{axis}")
                if axis == 0:
                    nc.vector.scalar_tensor_tensor(
                        out=c0[:, msl], in0=f0[:, msl], scalar=-2.0,
                        in1=cvb(1, mlen), op0=OP.max, op1=OP.min)
                else:
                    nc.vector.scalar_tensor_tensor(
                        out=c0[:, msl], in0=f0[:, msl],
                        scalar=float(PADT * PW + PADT),
                        in1=cvb(2, mlen), op0=OP.add, op1=OP.max)
                    nc.vector.tensor_tensor(
                        out=c0[:, msl], in0=c0[:, msl], in1=cvb(3, mlen),
                        op=OP.min)

            y_w0, y_w1 = pp("w0_0", BF16), pp("w1_0", BF16)
            x_w0, x_w1 = pp("w0_1", BF16), pp("w1_1", BF16)

            # wf[...,(i,j)] = wy_i * wx_j   (k-outer, corner-inner layout)
            for i, wy in enumerate((y_w0, y_w1)):
                for j, wx in enumerate((x_w0, x_w1)):
                    dst = wf_v[:, ysl, :, 2 * i + j].rearrange(
                        "p y k -> p (y k)")
                    nc.vector.tensor_tensor(
                        out=dst, in0=wy[:, msl], in1=wx[:, msl], op=OP.mult)

            # idx16 = yc*132 + (xc + 266), fused with the f32->i16 cast
            nc.vector.scalar_tensor_tensor(
                out=idx16[:, msl], in0=pp("c0_0")[:, msl], scalar=float(PW),
                in1=pp("c0_1")[:, msl], op0=OP.mult, op1=OP.add)

            # wrapped-16 idx layout for dma_gather: queue q's cpu pair reads
            # partitions 32q..32q+31, so each group's shuffles write ONLY its
            # queue's block (wrapped[32q+16h+p16, m*8+phi] = idx16[p16+16phi,
            # m]). No replication copies/DMAs: every gather's dependency set
            # stays DVE-only, which Tile expresses as one exact engine-tick
            # wait (multi-lane sets get merged into much later events).
            wrt = wr_pool.tile([128, max(secs) * KK * 8], I16, tag="wrapped")
            wrapped = wrt[:, 0:mlen * 8]
            wr_v = wrapped.rearrange("p (m e) -> p m e", m=mlen, e=8)
            GM = GROUP * KK
            for gg in range(srows // GROUP):
                q = 32 * ((gi0 + gg) % 4)
                msl_g = slice((row0 + gg * GROUP) * KK,
                              (row0 + (gg + 1) * GROUP) * KK)
                for qj in range(4):
                    for qh in range(2):
                        mask = [16 * qh + (p % 16) for p in range(32)]
                        nc.vector.stream_shuffle(
                            out=wr_v[q:q + 32, gg * GM:(gg + 1) * GM,
                                     2 * qj + qh],
                            in_=idx16[32 * qj:32 * (qj + 1), msl_g], mask=mask)

            # gathers for this section (queues round-robin)
            for gg in range(srows // GROUP):
                gi = gi0 + gg
                g = g_pool.tile([128, GROUP * KK * EROW], BF16, tag="g")
                gtiles[gi] = g
                g_m = g[:].rearrange("p (m e) -> p m e", m=GROUP * KK, e=EROW)
                nidx_g = 128 * GROUP * KK
                nc.gpsimd.dma_gather(
                    out_ap=g_m, in_ap=table[:],
                    idxs_ap=wrapped[:, gg * GROUP * KK * 8:
                                    (gg + 1) * GROUP * KK * 8],
                    num_idxs=nidx_g, num_idxs_reg=nidx_g, elem_size=EROW,
                    single_packet=False, queue_num=gi % 4)

            gi0 += srows // GROUP
            row0 += srows

        # all consumes after the full gather stream: keeps every gather's
        # DVE-tick wait below the consume ticks (no false serialization)
        for gi in range(nrows // GROUP):
            emit_consume(gi)

        ctx.close()
    nc.compile()
    return nc


# ---------------- host side ----------------

def make_constants():
    """Per-core constant tensors (identical across cores except cgridy)."""
    ky, kx = np.meshgrid(np.arange(3), np.arange(3), indexing="ij")
    ky = ky.reshape(KK).astype(np.float32)
    kx = kx.reshape(KK).astype(np.float32)
    return ky, kx


def host_prepare(x, off_w, off_b, weight, bias, nrows=ROWS):
    """Build per-core input maps."""
    B = x.shape[0]
    x = np.asarray(x, np.float32)
    # padded image for offset conv: [B, C, H+2, W+3]; xband2 stacks the
    # (dx=0) view on channels 0-63 and the (dx=1)-shifted view on 64-127.
    xpad = np.pad(x, ((0, 0), (0, 0), (1, 1), (1, 2))).astype(ml_dtypes.bfloat16)
    # patch table per image: padded-by-2 channels-last
    xp2 = np.pad(x, ((0, 0), (0, 0), (PADT, PADT + 1), (PADT, PADT + 1)))
    # [B, PW+1, PW+1, C] channels-last
    xcl = xp2.transpose(0, 2, 3, 1)
    # table[q=(yp*PW+xp)] = interleave over c: [x(y,x,c), x(y,x+1,c), x(y+1,x,c), x(y+1,x+1,c)]
    tables = []
    for b in range(B):
        t = np.empty((PW, PW, C, 4), np.float32)
        t[:, :, :, 0] = xcl[b, :PW, :PW]
        t[:, :, :, 1] = xcl[b, :PW, 1:PW + 1]
        t[:, :, :, 2] = xcl[b, 1:PW + 1, :PW]
        t[:, :, :, 3] = xcl[b, 1:PW + 1, 1:PW + 1]
        tables.append(t.reshape(NTAB, EROW).astype(ml_dtypes.bfloat16))

    # offset conv weights: channel perm [dy taps 0..8, dx taps 0..8]
    perm = [2 * k for k in range(KK)] + [2 * k + 1 for k in range(KK)]
    w_off_p = np.asarray(off_w, np.float32)[perm]          # [18, C, 3, 3]
    # lhsT per tap: tap k = dy*3+dx -> [C, 18]
    w_off_t = np.empty((KK, C, 18), np.float32)
    for k in range(KK):
        dy, dx = k // 3, k % 3
        w_off_t[k] = w_off_p[:, :, dy, dx].T               # [C, 18]
    # tap-pair lhsT: pair (dy,0) on channels 0-63 with (dy,1) on 64-127
    w_off_pair = np.empty((3, 128, 18), np.float32)
    for dy in range(3):
        w_off_pair[dy, :C] = w_off_t[3 * dy + 0]
        w_off_pair[dy, C:] = w_off_t[3 * dy + 1]
    w_off_t = w_off_t.astype(ml_dtypes.bfloat16)
    w_off_pair = w_off_pair.astype(ml_dtypes.bfloat16)
    b_off_p = np.asarray(off_b, np.float32)[perm].reshape(18, 1)

    # main weights: W2[(k,c,xe), o] = weight[o, c, k] duplicated per
    # x-corner pair (1152 = 9*128 rows exactly)
    wgt = np.asarray(weight, np.float32).reshape(O, C, KK)
    kc = wgt.transpose(2, 1, 0).reshape(KK * C, O)          # [(k,c), O]
    w2f = np.repeat(kc, 2, axis=0)                          # [(k,c,xe), O]
    w2f = w2f.reshape(KTILES, 128, O).astype(ml_dtypes.bfloat16)
    b2f = np.asarray(bias, np.float32).reshape(O, 1)

    ky, kx = make_constants()
    cvals = np.array([[BIG, 128.0, float(PADT * PW + PADT - 2),
                       float(PADT * PW + PADT + 128)]], np.float32)
    cone = np.ones((1, 1), ml_dtypes.bfloat16)

    in_maps = []
    for core in range(8):
        b, hh = core // 2, core % 2
        y0 = hh * 64
        rows = np.arange(y0, y0 + nrows, dtype=np.float32)
        cgy = (rows[:, None] + ky[None, :] - 1.0).reshape(1, nrows * KK)
        # x grid with the per-pixel iota folded in: cxfull[px, (y,k)] =
        # px + kx - 1
        px = np.arange(128, dtype=np.float32)
        cgx = np.broadcast_to(kx[None, :] - 1.0, (nrows, KK)).reshape(nrows * KK)
        cxfull = (px[:, None] + cgx[None, :]).astype(ml_dtypes.bfloat16)
        band = xpad[b, :, y0:y0 + nrows + 2, :]             # [C, nrows+2, XBW]
        xband2 = np.concatenate([band, np.roll(band, -1, axis=2)], axis=0)
        in_maps.append({
            "xband2": np.ascontiguousarray(xband2),
            "table": tables[b],
            "w_off": w_off_t,
            "w_offp": w_off_pair,
            "b_off": b_off_p,
            "w2": w2f,
            "b2": b2f,
            "cgridy": cgy.astype(ml_dtypes.bfloat16),
            "cxfull": np.ascontiguousarray(cxfull),
            "cvals": cvals,
            "cone": cone,
        })
    return in_maps


def assemble(outs, nrows=ROWS):
    """outs: list of 8 dicts with 'out' [O, nrows*W] -> [4, O, H, W]"""
    full = np.zeros((4, O, H, W), np.float32)
    for core, om in enumerate(outs):
        b, hh = core // 2, core % 2
        full[b, :, hh * 64:hh * 64 + nrows] = om["out"].reshape(O, nrows, W)
    return full


_CACHE = {}


def kernel(x, off_w, off_b, weight, bias):
    if "nc" not in _CACHE:
        _CACHE["nc"] = build_program()
    nc = _CACHE["nc"]
    in_maps = host_prepare(x, off_w, off_b, weight, bias)
    from concourse.bass_utils import run_bass_kernel_spmd
    res = run_bass_kernel_spmd(nc, in_maps, core_ids=list(range(8)))
    return assemble(res.results)


# revision 14
# speedup vs baseline: 1.0150x; 1.0150x over previous
"""Trainium2 Bass kernel for DeformableConvBlock (B=4, C=64, H=W=128, K=3).

Self-contained: builds an SPMD Bass/Tile program for 8 NeuronCores.
Core c handles image c//2, output-row half c%2 (data-parallel over
batch x row-halves).

v3 pipeline. Key constraint discovered on HW: the GpSimd Q7 SWDGE
descriptor generation (dma_gather) shares an SBUF port pair with the
DVE's 2-port perf modes, so any tensor_scalar/tensor_copy/memset issued
while a gather is generating blocks until that generation finishes
(~36us). The steady-state therefore uses ONLY contention-free DVE ops
(tensor_tensor / scalar_tensor_tensor / stream_shuffle); index
replication runs as SBUF->SBUF HWDGE DMAs, and the s8 pad columns live
in two persistent buffers zeroed once up front.

Front end per row-section ([8,8,16,16,16]): offset-conv (PE) ->
DMA-transpose to pixel-major -> fused bilinear weight/index prep (DVE)
-> wrapped-index stream_shuffles + replicate DMAs -> dma_gathers
(4608 idx x 512B each, queues round-robin; 4 queues generate
concurrently at ~7.9ns/idx -> 146us floor for 16 gathers).
Bilinear consumes (DVE mult + 2 adds), transpose, PE contraction, bias,
out-DMA are interleaved a few sections behind the gather stream.

kernel(**inputs) takes the full unsharded numpy inputs and returns the
full [4, 64, 128, 128] float32 output.
"""
from contextlib import ExitStack

import numpy as np
import ml_dtypes

import concourse.bacc as bacc
import concourse.bass as bass
import concourse.mybir as mybir
import concourse.tile as tile
from concourse.tile import TileContext
from concourse.vector_clock import ScopedClock, VectorClock

F32 = mybir.dt.float32
BF16 = mybir.dt.bfloat16
I32 = mybir.dt.int32
I16 = mybir.dt.int16
AF = mybir.ActivationFunctionType
OP = mybir.AluOpType

H = W = 128
C = 64
O = 64
KK = 9
ROWS = 64            # output rows per core
PADT = 2             # table padding on each side
PW = W + 2 * PADT    # 132
NTAB = PW * PW       # 17424 table rows
EROW = 4 * C         # 256 elems per table row
KTILES = 9           # 1152 = 9*128 contraction tiles: (k, c, x-corner pair)
GROUP = 4            # rows per gather group = rows per matmul batch
XBW = W + 3          # xband2 column count
BIG = 12582912.0     # 1.5 * 2^23 fp32 rounding offset
SECS = [4, 12, 16, 16, 16]


class TileContextSplitDrain(TileContext):
    """Stock epilogue emits one Drain with one wait per outstanding proc;
    this walrus rejects >1 sync wait per instruction, so emit one Drain
    per proc instead."""

    def _drain_and_barrier(self, tick_clock, wait_clock):
        gc = tick_clock.global_clock
        nprocs = len(gc)
        emitted = False
        for p in range(nprocs):
            t = gc[p]
            if t <= 0:
                continue
            vec = [0] * nprocs
            vec[p] = t
            drain_inst = self.nc.sync.drain()
            wait_clock.add_sem_waits(
                drain_inst.ins, ScopedClock({None: VectorClock(vec)})
            )
            si = drain_inst.ins.sync_info
            assert si is None or len(si.on_wait) <= 1
            emitted = True
        if not emitted:
            self.nc.sync.drain()
        self.nc.all_engine_barrier()
        assert self.sems is not None
        popped = self.nc._tile_sem_poison_stack.pop()
        assert popped is self._sem_poison
        self.nc.clear_and_free_semaphores(list(self.sems.allocated().values()))
        self.nc.all_engine_barrier()


def build_program(nrows=ROWS, g_bufs=6):
    npx = nrows * W
    secs = SECS if nrows >= 64 else [nrows]

    nc = bacc.Bacc(num_swdge_queues=4)
    # xband2: rows 0-63 channel c of padded x (cols 0..XBW-1), rows 64-127
    # the same shifted one column left (cols 1..XBW).
    xband2 = nc.dram_tensor("xband2", [128, nrows + 2, XBW], BF16, kind="ExternalInput")
    table = nc.dram_tensor("table", [NTAB, EROW], BF16, kind="ExternalInput")
    w_off = nc.dram_tensor("w_off", [KK, C, 18], BF16, kind="ExternalInput")
    w_offp = nc.dram_tensor("w_offp", [3, 128, 18], BF16, kind="ExternalInput")
    b_off = nc.dram_tensor("b_off", [18, 1], F32, kind="ExternalInput")
    w2 = nc.dram_tensor("w2", [KTILES, 128, O], BF16, kind="ExternalInput")
    b2 = nc.dram_tensor("b2", [O, 1], F32, kind="ExternalInput")
    cgridy = nc.dram_tensor("cgridy", [1, nrows * KK], BF16, kind="ExternalInput")
    cxfull = nc.dram_tensor("cxfull", [128, nrows * KK], BF16, kind="ExternalInput")
    cvals = nc.dram_tensor("cvals", [1, 4], F32, kind="ExternalInput")
    cone = nc.dram_tensor("cone", [1, 1], BF16, kind="ExternalInput")
    out = nc.dram_tensor("out", [O, npx], F32, kind="ExternalOutput")

    ctx = ExitStack()
    with TileContextSplitDrain(nc) as tc:
        # wrapped-idx tiles only fill the 32-partition block the gather's
        # queue actually reads; other blocks keep stale (in-bounds) data
        # that the race detector would flag
        tc.race_detector_enabled = False
        const_pool = ctx.enter_context(tc.tile_pool(name="const", bufs=1))
        big_pool = ctx.enter_context(tc.tile_pool(name="big", bufs=1))
        prep_pool = ctx.enter_context(tc.tile_pool(name="prep", bufs=1))
        oc_pool = ctx.enter_context(tc.tile_pool(name="oc", bufs=2))
        wr_pool = ctx.enter_context(tc.tile_pool(name="wr", bufs=5))
        g_pool = ctx.enter_context(tc.tile_pool(name="g", bufs=g_bufs))
        st_pool = ctx.enter_context(tc.tile_pool(name="st", bufs=2))
        o_pool = ctx.enter_context(tc.tile_pool(name="o", bufs=2))
        psum_pool = ctx.enter_context(tc.tile_pool(name="ps", bufs=2, space="PSUM"))
        psum2_pool = ctx.enter_context(tc.tile_pool(name="ps2", bufs=2, space="PSUM"))

        # ---- constant loads ----
        xb = const_pool.tile([128, (nrows + 2) * XBW], BF16)
        xb_v = xb[:].rearrange("c (h w) -> c h w", h=nrows + 2, w=XBW)
        # split band load so the first offset-conv section starts early
        xsplit = min(secs[0] + 6, nrows + 2)
        nc.sync.dma_start(out=xb_v[:, 0:xsplit], in_=xband2[:, 0:xsplit])
        nc.sync.dma_start(out=xb_v[:, xsplit:], in_=xband2[:, xsplit:])

        wof = const_pool.tile([C, KK * 18], BF16)
        wof_v = wof[:].rearrange("c (k e) -> c k e", k=KK, e=18)
        nc.sync.dma_start(out=wof_v, in_=w_off[:].rearrange("k c e -> c k e"))

        wofp = const_pool.tile([128, 3 * 18], BF16)
        wofp_v = wofp[:].rearrange("c (k e) -> c k e", k=3, e=18)
        nc.sync.dma_start(out=wofp_v, in_=w_offp[:].rearrange("k c e -> c k e"))

        bof = const_pool.tile([18, 1], F32)
        nc.sync.dma_start(out=bof[:], in_=b_off[:])

        w2t = const_pool.tile([128, KTILES * O], BF16)
        w2t_v = w2t[:].rearrange("p (j e) -> p j e", j=KTILES, e=O)
        nc.sync.dma_start(out=w2t_v, in_=w2[:].rearrange("j p e -> p j e"))

        b2t = const_pool.tile([O, 1], F32)
        nc.sync.dma_start(out=b2t[:], in_=b2[:])

        cy = const_pool.tile([128, nrows * KK], BF16)
        nc.sync.dma_start(out=cy[:], in_=cgridy[0:1, :].to_broadcast((128, nrows * KK)))
        cxf = const_pool.tile([128, nrows * KK], BF16)
        nc.sync.dma_start(out=cxf[:], in_=cxfull[:])
        cv = const_pool.tile([128, 4], F32)
        nc.sync.dma_start(out=cv[:], in_=cvals[0:1, :].to_broadcast((128, 4)))
        one_b = const_pool.tile([128, 1], BF16)
        nc.sync.dma_start(out=one_b[:], in_=cone[0:1, :].to_broadcast((128, 1)))

        def cvb(i, n):  # broadcast view of constant column i
            return cv[:, i:i + 1].to_broadcast((128, n))

        # ---- persistent whole-half tiles ----
        op = big_pool.tile([128, nrows * 32], BF16)
        op3 = op[:].rearrange("p (y e) -> p y e", y=nrows, e=32)
        wf = big_pool.tile([128, nrows * 36], BF16)
        wf_v = wf[:].rearrange("p (y k c) -> p y k c", y=nrows, k=KK, c=4)
        idx16 = big_pool.tile([128, nrows * KK], I16)
        # two persistent s8 buffers (1152 = 9*64*2 used exactly, no padding)
        s8ab = [
            big_pool.tile([128, GROUP * KTILES * 128], BF16,
                          tag=f"s8_{sbi}", name=f"s8_{sbi}")
            for sbi in range(2)
        ]

        ptiles = {}

        max_mlen = max(secs) * KK

        def pp(tag, dt=F32):
            if tag not in ptiles:
                ptiles[tag] = prep_pool.tile(
                    [128, max_mlen], dt, tag=tag, name=f"prep_{tag}")
            return ptiles[tag]

        gtiles = [None] * (nrows // GROUP)

        def emit_consume(gi):
            y0 = gi * GROUP
            g = gtiles[gi]
            g_v = g[:].rearrange(
                "p (y k c e) -> p y k c e", y=GROUP, k=KK, c=C, e=4)
            wfb = wf_v[:, y0:y0 + GROUP, :, None, :].to_broadcast(
                (128, GROUP, KK, C, 4))
            nc.vector.tensor_tensor(out=g_v, in0=g_v, in1=wfb, op=OP.mult)
            # y-corner pairwise add, compacted into s8: s8[y,k,c,xe] =
            # g[...,xe] + g[...,xe+2]; the x-corner pair stays in the PE
            # contraction (weights duplicated per pair)
            s8 = s8ab[gi % 2]
            s8_v = s8[:].rearrange(
                "p (y k c e) -> p y k c e", y=GROUP, k=KK, c=C, e=2)
            nc.vector.tensor_tensor(
                out=s8_v, in0=g_v[:, :, :, :, 0:2],
                in1=g_v[:, :, :, :, 2:4], op=OP.add)

            st = st_pool.tile([128, GROUP * KTILES * 128], BF16, tag="st")
            st_v = st[:].rearrange(
                "p (m c) -> p m c", m=GROUP * KTILES, c=128)
            nc.scalar.dma_start_transpose(out=st_v, in_=s8[:])
            st_y = st[:].rearrange(
                "p (y j c) -> p y j c", y=GROUP, j=KTILES, c=128)
            ps2 = psum2_pool.tile([O, 512], F32, tag="ps2")
            for j in range(KTILES):
                nc.tensor.matmul(
                    out=ps2[:], lhsT=w2t_v[:, j, :], rhs=st_y[:, :, j, :],
                    start=(j == 0), stop=(j == KTILES - 1),
                )
            ob = o_pool.tile([O, 512], F32, tag="ob")
            nc.scalar.activation(
                out=ob[:], in_=ps2[:], func=AF.Identity, bias=b2t[:])
            nc.scalar.dma_start(
                out=out[:, gi * 512:(gi + 1) * 512], in_=ob[:])

        # ---------------- sections ----------------
        row0 = 0
        gi0 = 0
        for si, srows in enumerate(secs):
            ysl = slice(row0, row0 + srows)
            msl = slice(row0 * KK, (row0 + srows) * KK)
            mlen = srows * KK

            # offset conv: per 4-row chunk, 3 tap-pair matmuls
            # (contraction 128 = taps (dy,0)+(dy,1)) + 3 singles (dy,2)
            oct_ = oc_pool.tile([32, max(secs) * W], BF16, tag="ocq")
            ocq = oct_[:, 0:srows * W]
            if si < 2:  # oc buffers only ever written on rows 0:18; zero once
                nc.scalar.memzero(oct_[:])
            for cc in range(srows // 4):
                r0 = row0 + cc * 4
                ps = psum_pool.tile([18, 512], F32, tag="ps1")
                for dy in range(3):
                    nc.tensor.matmul(
                        out=ps[:], lhsT=wofp_v[:, dy, :],
                        rhs=xb_v[:, r0 + dy:r0 + dy + 4, 0:W],
                        start=(dy == 0), stop=False,
                    )
                for dy in range(3):
                    nc.tensor.matmul(
                        out=ps[:], lhsT=wof_v[:, 3 * dy + 2, :],
                        rhs=xb_v[0:C, r0 + dy:r0 + dy + 4, 2:2 + W],
                        start=False, stop=(dy == 2),
                    )
                nc.scalar.activation(
                    out=ocq[:18, cc * 512:(cc + 1) * 512], in_=ps[:],
                    func=AF.Identity, bias=bof[:],
                )

            # offsets to pixel-major
            op3q = op3[:, ysl, :]
            nc.sync.dma_start_transpose(out=op3q, in_=ocq)

            # ---- wf/idx prep: contention-free ops only (TT / STT) ----
            # prep scratch tiles are section-sized; lsl is the local slice
            s = pp("s")
            f0 = pp("f0")
            lt = pp("lt")
            lsl = slice(0, mlen)
            s3 = s[:, lsl].rearrange("p (y k) -> p y k", y=srows, k=KK)
            for axis in (0, 1):  # 0: y, 1: x
                grid = cy if axis == 0 else cxf
                grid3 = grid[:].rearrange("p (y k) -> p y k", y=nrows, k=KK)
                nc.vector.tensor_tensor(
                    out=s3, in0=op3q[:, :, axis * 9:axis * 9 + 9],
                    in1=grid3[:, ysl], op=OP.add)
                # round to nearest via +-1.5*2^23 (stage0 scalar add rounds,
                # stage1 tensor subtract undoes; no constant folding across
                # ALU stages), then floor = round - (s < round)
                nc.vector.scalar_tensor_tensor(
                    out=f0[:, lsl], in0=s[:, lsl], scalar=BIG,
                    in1=cvb(0, mlen), op0=OP.add, op1=OP.subtract)
                nc.vector.tensor_tensor(
                    out=lt[:, lsl], in0=s[:, lsl], in1=f0[:, lsl], op=OP.is_lt)
                nc.vector.tensor_tensor(
                    out=f0[:, lsl], in0=f0[:, lsl], in1=lt[:, lsl],
                    op=OP.subtract)
                # fractions
                w1 = pp(f"w1_{axis}", BF16)
                nc.vector.tensor_tensor(
                    out=w1[:, lsl], in0=s[:, lsl], in1=f0[:, lsl],
                    op=OP.subtract)
                w0 = pp(f"w0_{axis}", BF16)
                nc.vector.scalar_tensor_tensor(
                    out=w0[:, lsl], in0=w1[:, lsl], scalar=-1.0,
                    in1=one_b[:].to_broadcast((128, mlen)),
                    op0=OP.mult, op1=OP.add)
                # clamp for index (clamped coords land in the zero pad);
                # x additionally folds in the +266 table offset
                c0 = pp(f"c0_{axis}")
                if axis == 0:
                    nc.vector.scalar_tensor_tensor(
                        out=c0[:, lsl], in0=f0[:, lsl], scalar=-2.0,
                        in1=cvb(1, mlen), op0=OP.max, op1=OP.min)
                else:
                    nc.vector.scalar_tensor_tensor(
                        out=c0[:, lsl], in0=f0[:, lsl],
                        scalar=float(PADT * PW + PADT),
                        in1=cvb(2, mlen), op0=OP.add, op1=OP.max)
                    nc.vector.tensor_tensor(
                        out=c0[:, lsl], in0=c0[:, lsl], in1=cvb(3, mlen),
                        op=OP.min)

            y_w0, y_w1 = pp("w0_0", BF16), pp("w1_0", BF16)
            x_w0, x_w1 = pp("w0_1", BF16), pp("w1_1", BF16)

            # wf[...,(i,j)] = wy_i * wx_j   (k-outer, corner-inner layout)
            for i, wy in enumerate((y_w0, y_w1)):
                for j, wx in enumerate((x_w0, x_w1)):
                    dst = wf_v[:, ysl, :, 2 * i + j].rearrange(
                        "p y k -> p (y k)")
                    nc.vector.tensor_tensor(
                        out=dst, in0=wy[:, lsl], in1=wx[:, lsl], op=OP.mult)

            # idx16 = yc*132 + (xc + 266), fused with the f32->i16 cast
            nc.vector.scalar_tensor_tensor(
                out=idx16[:, msl], in0=pp("c0_0")[:, lsl], scalar=float(PW),
                in1=pp("c0_1")[:, lsl], op0=OP.mult, op1=OP.add)

            # wrapped-16 idx layout for dma_gather: queue q's cpu pair reads
            # partitions 32q..32q+31, so each group's shuffles write ONLY its
            # queue's block (wrapped[32q+16h+p16, m*8+phi] = idx16[p16+16phi,
            # m]). No replication copies/DMAs: every gather's dependency set
            # stays DVE-only, which Tile expresses as one exact engine-tick
            # wait (multi-lane sets get merged into much later events).
            wrt = wr_pool.tile([128, max(secs) * KK * 8], I16, tag="wrapped")
            wrapped = wrt[:, 0:mlen * 8]
            wr_v = wrapped.rearrange("p (m e) -> p m e", m=mlen, e=8)
            GM = GROUP * KK
            for gg in range(srows // GROUP):
                q = 32 * ((gi0 + gg) % 4)
                msl_g = slice((row0 + gg * GROUP) * KK,
                              (row0 + (gg + 1) * GROUP) * KK)
                for qj in range(4):
                    for qh in range(2):
                        mask = [16 * qh + (p % 16) for p in range(32)]
                        nc.vector.stream_shuffle(
                            out=wr_v[q:q + 32, gg * GM:(gg + 1) * GM,
                                     2 * qj + qh],
                            in_=idx16[32 * qj:32 * (qj + 1), msl_g], mask=mask)

            # gathers for this section (queues round-robin)
            for gg in range(srows // GROUP):
                gi = gi0 + gg
                g = g_pool.tile([128, GROUP * KK * EROW], BF16, tag="g")
                gtiles[gi] = g
                g_m = g[:].rearrange("p (m e) -> p m e", m=GROUP * KK, e=EROW)
                nidx_g = 128 * GROUP * KK
                nc.gpsimd.dma_gather(
                    out_ap=g_m, in_ap=table[:],
                    idxs_ap=wrapped[:, gg * GROUP * KK * 8:
                                    (gg + 1) * GROUP * KK * 8],
                    num_idxs=nidx_g, num_idxs_reg=nidx_g, elem_size=EROW,
                    single_packet=False, queue_num=gi % 4)

            gi0 += srows // GROUP
            row0 += srows

        # all consumes after the full gather stream: keeps every gather's
        # DVE-tick wait below the consume ticks (no false serialization)
        for gi in range(nrows // GROUP):
            emit_consume(gi)

        ctx.close()
    nc.compile()
    return nc


# ---------------- host side ----------------

def make_constants():
    """Per-core constant tensors (identical across cores except cgridy)."""
    ky, kx = np.meshgrid(np.arange(3), np.arange(3), indexing="ij")
    ky = ky.reshape(KK).astype(np.float32)
    kx = kx.reshape(KK).astype(np.float32)
    return ky, kx


def host_prepare(x, off_w, off_b, weight, bias, nrows=ROWS):
    """Build per-core input maps."""
    B = x.shape[0]
    x = np.asarray(x, np.float32)
    # padded image for offset conv: [B, C, H+2, W+3]; xband2 stacks the
    # (dx=0) view on channels 0-63 and the (dx=1)-shifted view on 64-127.
    xpad = np.pad(x, ((0, 0), (0, 0), (1, 1), (1, 2))).astype(ml_dtypes.bfloat16)
    # patch table per image: padded-by-2 channels-last
    xp2 = np.pad(x, ((0, 0), (0, 0), (PADT, PADT + 1), (PADT, PADT + 1)))
    # [B, PW+1, PW+1, C] channels-last
    xcl = xp2.transpose(0, 2, 3, 1)
    # table[q=(yp*PW+xp)] = interleave over c: [x(y,x,c), x(y,x+1,c), x(y+1,x,c), x(y+1,x+1,c)]
    tables = []
    for b in range(B):
        t = np.empty((PW, PW, C, 4), np.float32)
        t[:, :, :, 0] = xcl[b, :PW, :PW]
        t[:, :, :, 1] = xcl[b, :PW, 1:PW + 1]
        t[:, :, :, 2] = xcl[b, 1:PW + 1, :PW]
        t[:, :, :, 3] = xcl[b, 1:PW + 1, 1:PW + 1]
        tables.append(t.reshape(NTAB, EROW).astype(ml_dtypes.bfloat16))

    # offset conv weights: channel perm [dy taps 0..8, dx taps 0..8]
    perm = [2 * k for k in range(KK)] + [2 * k + 1 for k in range(KK)]
    w_off_p = np.asarray(off_w, np.float32)[perm]          # [18, C, 3, 3]
    # lhsT per tap: tap k = dy*3+dx -> [C, 18]
    w_off_t = np.empty((KK, C, 18), np.float32)
    for k in range(KK):
        dy, dx = k // 3, k % 3
        w_off_t[k] = w_off_p[:, :, dy, dx].T               # [C, 18]
    # tap-pair lhsT: pair (dy,0) on channels 0-63 with (dy,1) on 64-127
    w_off_pair = np.empty((3, 128, 18), np.float32)
    for dy in range(3):
        w_off_pair[dy, :C] = w_off_t[3 * dy + 0]
        w_off_pair[dy, C:] = w_off_t[3 * dy + 1]
    w_off_t = w_off_t.astype(ml_dtypes.bfloat16)
    w_off_pair = w_off_pair.astype(ml_dtypes.bfloat16)
    b_off_p = np.asarray(off_b, np.float32)[perm].reshape(18, 1)

    # main weights: W2[(k,c,xe), o] = weight[o, c, k] duplicated per
    # x-corner pair (1152 = 9*128 rows exactly)
    wgt = np.asarray(weight, np.float32).reshape(O, C, KK)
    kc = wgt.transpose(2, 1, 0).reshape(KK * C, O)          # [(k,c), O]
    w2f = np.repeat(kc, 2, axis=0)                          # [(k,c,xe), O]
    w2f = w2f.reshape(KTILES, 128, O).astype(ml_dtypes.bfloat16)
    b2f = np.asarray(bias, np.float32).reshape(O, 1)

    ky, kx = make_constants()
    cvals = np.array([[BIG, 128.0, float(PADT * PW + PADT - 2),
                       float(PADT * PW + PADT + 128)]], np.float32)
    cone = np.ones((1, 1), ml_dtypes.bfloat16)

    in_maps = []
    for core in range(8):
        b, hh = core // 2, core % 2
        y0 = hh * 64
        rows = np.arange(y0, y0 + nrows, dtype=np.float32)
        cgy = (rows[:, None] + ky[None, :] - 1.0).reshape(1, nrows * KK)
        # x grid with the per-pixel iota folded in: cxfull[px, (y,k)] =
        # px + kx - 1
        px = np.arange(128, dtype=np.float32)
        cgx = np.broadcast_to(kx[None, :] - 1.0, (nrows, KK)).reshape(nrows * KK)
        cxfull = (px[:, None] + cgx[None, :]).astype(ml_dtypes.bfloat16)
        band = xpad[b, :, y0:y0 + nrows + 2, :]             # [C, nrows+2, XBW]
        xband2 = np.concatenate([band, np.roll(band, -1, axis=2)], axis=0)
        in_maps.append({
            "xband2": np.ascontiguousarray(xband2),
            "table": tables[b],
            "w_off": w_off_t,
            "w_offp": w_off_pair,
            "b_off": b_off_p,
            "w2": w2f,
            "b2": b2f,
            "cgridy": cgy.astype(ml_dtypes.bfloat16),
            "cxfull": np.ascontiguousarray(cxfull),
            "cvals": cvals,
            "cone": cone,
        })
    return in_maps


def assemble(outs, nrows=ROWS):
    """outs: list of 8 dicts with 'out' [O, nrows*W] -> [4, O, H, W]"""
    full = np.zeros((4, O, H, W), np.float32)
    for core, om in enumerate(outs):
        b, hh = core // 2, core % 2
        full[b, :, hh * 64:hh * 64 + nrows] = om["out"].reshape(O, nrows, W)
    return full


_CACHE = {}


def kernel(x, off_w, off_b, weight, bias):
    if "nc" not in _CACHE:
        _CACHE["nc"] = build_program()
    nc = _CACHE["nc"]
    in_maps = host_prepare(x, off_w, off_b, weight, bias)
    from concourse.bass_utils import run_bass_kernel_spmd
    res = run_bass_kernel_spmd(nc, in_maps, core_ids=list(range(8)))
    return assemble(res.results)


# revision 15
# speedup vs baseline: 1.0435x; 1.0281x over previous
"""Trainium2 Bass kernel for DeformableConvBlock (B=4, C=64, H=W=128, K=3).

Self-contained: builds an SPMD Bass/Tile program for 8 NeuronCores.
Core c handles image c//2, output-row half c%2 (data-parallel over
batch x row-halves).

v3 pipeline. Key constraint discovered on HW: the GpSimd Q7 SWDGE
descriptor generation (dma_gather) shares an SBUF port pair with the
DVE's 2-port perf modes, so any tensor_scalar/tensor_copy/memset issued
while a gather is generating blocks until that generation finishes
(~36us). The steady-state therefore uses ONLY contention-free DVE ops
(tensor_tensor / scalar_tensor_tensor / stream_shuffle); index
replication runs as SBUF->SBUF HWDGE DMAs, and the s8 pad columns live
in two persistent buffers zeroed once up front.

Front end per row-section ([8,8,16,16,16]): offset-conv (PE) ->
DMA-transpose to pixel-major -> fused bilinear weight/index prep (DVE)
-> wrapped-index stream_shuffles + replicate DMAs -> dma_gathers
(4608 idx x 512B each, queues round-robin; 4 queues generate
concurrently at ~7.9ns/idx -> 146us floor for 16 gathers).
Bilinear consumes (DVE mult + 2 adds), transpose, PE contraction, bias,
out-DMA are interleaved a few sections behind the gather stream.

kernel(**inputs) takes the full unsharded numpy inputs and returns the
full [4, 64, 128, 128] float32 output.
"""
from contextlib import ExitStack

import numpy as np
import ml_dtypes

import concourse.bacc as bacc
import concourse.bass as bass
import concourse.mybir as mybir
import concourse.tile as tile
from concourse.tile import TileContext
from concourse.vector_clock import ScopedClock, VectorClock

F32 = mybir.dt.float32
BF16 = mybir.dt.bfloat16
I32 = mybir.dt.int32
I16 = mybir.dt.int16
AF = mybir.ActivationFunctionType
OP = mybir.AluOpType

H = W = 128
C = 64
O = 64
KK = 9
ROWS = 64            # output rows per core
PADT = 2             # table padding on each side
PW = W + 2 * PADT    # 132
NTAB = PW * PW       # 17424 table rows
EROW = 4 * C         # 256 elems per table row
KTILES = 9           # 1152 = 9*128 contraction tiles: (k, c, x-corner pair)
GROUP = 2            # rows per gather group = rows per matmul batch
XBW = W + 3          # xband2 column count
BIG = 12582912.0     # 1.5 * 2^23 fp32 rounding offset
SECS = [4, 12, 16, 16, 16]


class TileContextSplitDrain(TileContext):
    """Stock epilogue emits one Drain with one wait per outstanding proc;
    this walrus rejects >1 sync wait per instruction, so emit one Drain
    per proc instead."""

    def _drain_and_barrier(self, tick_clock, wait_clock):
        gc = tick_clock.global_clock
        nprocs = len(gc)
        emitted = False
        for p in range(nprocs):
            t = gc[p]
            if t <= 0:
                continue
            vec = [0] * nprocs
            vec[p] = t
            drain_inst = self.nc.sync.drain()
            wait_clock.add_sem_waits(
                drain_inst.ins, ScopedClock({None: VectorClock(vec)})
            )
            si = drain_inst.ins.sync_info
            assert si is None or len(si.on_wait) <= 1
            emitted = True
        if not emitted:
            self.nc.sync.drain()
        self.nc.all_engine_barrier()
        assert self.sems is not None
        popped = self.nc._tile_sem_poison_stack.pop()
        assert popped is self._sem_poison
        self.nc.clear_and_free_semaphores(list(self.sems.allocated().values()))
        self.nc.all_engine_barrier()


def build_program(nrows=ROWS, g_bufs=13):
    npx = nrows * W
    secs = SECS if nrows >= 64 else [nrows]

    nc = bacc.Bacc(num_swdge_queues=4)
    # xband2: rows 0-63 channel c of padded x (cols 0..XBW-1), rows 64-127
    # the same shifted one column left (cols 1..XBW).
    xband2 = nc.dram_tensor("xband2", [128, nrows + 2, XBW], BF16, kind="ExternalInput")
    table = nc.dram_tensor("table", [NTAB, EROW], BF16, kind="ExternalInput")
    w_off = nc.dram_tensor("w_off", [KK, C, 18], BF16, kind="ExternalInput")
    w_offp = nc.dram_tensor("w_offp", [3, 128, 18], BF16, kind="ExternalInput")
    b_off = nc.dram_tensor("b_off", [18, 1], F32, kind="ExternalInput")
    w2 = nc.dram_tensor("w2", [KTILES, 128, O], BF16, kind="ExternalInput")
    b2 = nc.dram_tensor("b2", [O, 1], F32, kind="ExternalInput")
    cgridy = nc.dram_tensor("cgridy", [1, nrows * KK], BF16, kind="ExternalInput")
    cxfull = nc.dram_tensor("cxfull", [128, nrows * KK], BF16, kind="ExternalInput")
    cvals = nc.dram_tensor("cvals", [1, 4], F32, kind="ExternalInput")
    cone = nc.dram_tensor("cone", [1, 1], BF16, kind="ExternalInput")
    out = nc.dram_tensor("out", [O, npx], F32, kind="ExternalOutput")

    ctx = ExitStack()
    with TileContextSplitDrain(nc) as tc:
        # wrapped-idx tiles only fill the 32-partition block the gather's
        # queue actually reads; other blocks keep stale (in-bounds) data
        # that the race detector would flag
        tc.race_detector_enabled = False
        const_pool = ctx.enter_context(tc.tile_pool(name="const", bufs=1))
        big_pool = ctx.enter_context(tc.tile_pool(name="big", bufs=1))
        prep_pool = ctx.enter_context(tc.tile_pool(name="prep", bufs=1))
        oc_pool = ctx.enter_context(tc.tile_pool(name="oc", bufs=2))
        wr_pool = ctx.enter_context(tc.tile_pool(name="wr", bufs=5))
        g_pool = ctx.enter_context(tc.tile_pool(name="g", bufs=g_bufs))
        st_pool = ctx.enter_context(tc.tile_pool(name="st", bufs=2))
        o_pool = ctx.enter_context(tc.tile_pool(name="o", bufs=2))
        psum_pool = ctx.enter_context(tc.tile_pool(name="ps", bufs=2, space="PSUM"))
        psum2_pool = ctx.enter_context(tc.tile_pool(name="ps2", bufs=2, space="PSUM"))

        # ---- constant loads ----
        xb = const_pool.tile([128, (nrows + 2) * XBW], BF16)
        xb_v = xb[:].rearrange("c (h w) -> c h w", h=nrows + 2, w=XBW)
        # split band load so the first offset-conv section starts early
        xsplit = min(secs[0] + 6, nrows + 2)
        nc.sync.dma_start(out=xb_v[:, 0:xsplit], in_=xband2[:, 0:xsplit])
        nc.sync.dma_start(out=xb_v[:, xsplit:], in_=xband2[:, xsplit:])

        wof = const_pool.tile([C, KK * 18], BF16)
        wof_v = wof[:].rearrange("c (k e) -> c k e", k=KK, e=18)
        nc.sync.dma_start(out=wof_v, in_=w_off[:].rearrange("k c e -> c k e"))

        wofp = const_pool.tile([128, 3 * 18], BF16)
        wofp_v = wofp[:].rearrange("c (k e) -> c k e", k=3, e=18)
        nc.sync.dma_start(out=wofp_v, in_=w_offp[:].rearrange("k c e -> c k e"))

        bof = const_pool.tile([18, 1], F32)
        nc.sync.dma_start(out=bof[:], in_=b_off[:])

        w2t = const_pool.tile([128, KTILES * O], BF16)
        w2t_v = w2t[:].rearrange("p (j e) -> p j e", j=KTILES, e=O)
        nc.sync.dma_start(out=w2t_v, in_=w2[:].rearrange("j p e -> p j e"))

        b2t = const_pool.tile([O, 1], F32)
        nc.sync.dma_start(out=b2t[:], in_=b2[:])

        cy = const_pool.tile([128, nrows * KK], BF16)
        nc.sync.dma_start(out=cy[:], in_=cgridy[0:1, :].to_broadcast((128, nrows * KK)))
        cxf = const_pool.tile([128, nrows * KK], BF16)
        nc.sync.dma_start(out=cxf[:], in_=cxfull[:])
        cv = const_pool.tile([128, 4], F32)
        nc.sync.dma_start(out=cv[:], in_=cvals[0:1, :].to_broadcast((128, 4)))
        one_b = const_pool.tile([128, 1], BF16)
        nc.sync.dma_start(out=one_b[:], in_=cone[0:1, :].to_broadcast((128, 1)))

        def cvb(i, n):  # broadcast view of constant column i
            return cv[:, i:i + 1].to_broadcast((128, n))

        # ---- persistent whole-half tiles ----
        op = big_pool.tile([128, nrows * 32], BF16)
        op3 = op[:].rearrange("p (y e) -> p y e", y=nrows, e=32)
        wf = big_pool.tile([128, nrows * 36], BF16)
        wf_v = wf[:].rearrange("p (y k c) -> p y k c", y=nrows, k=KK, c=4)
        idx16 = big_pool.tile([128, nrows * KK], I16)
        # two persistent s8 buffers (1152 = 9*64*2 used exactly, no padding)
        s8ab = [
            big_pool.tile([128, GROUP * KTILES * 128], BF16,
                          tag=f"s8_{sbi}", name=f"s8_{sbi}")
            for sbi in range(2)
        ]

        ptiles = {}

        max_mlen = max(secs) * KK

        def pp(tag, dt=F32):
            if tag not in ptiles:
                ptiles[tag] = prep_pool.tile(
                    [128, max_mlen], dt, tag=tag, name=f"prep_{tag}")
            return ptiles[tag]

        gtiles = [None] * (nrows // GROUP)

        def emit_consume(gi):
            y0 = gi * GROUP
            g = gtiles[gi]
            g_v = g[:].rearrange(
                "p (y k c e) -> p y k c e", y=GROUP, k=KK, c=C, e=4)
            wfb = wf_v[:, y0:y0 + GROUP, :, None, :].to_broadcast(
                (128, GROUP, KK, C, 4))
            nc.vector.tensor_tensor(out=g_v, in0=g_v, in1=wfb, op=OP.mult)
            # y-corner pairwise add, compacted into s8: s8[y,k,c,xe] =
            # g[...,xe] + g[...,xe+2]; the x-corner pair stays in the PE
            # contraction (weights duplicated per pair)
            s8 = s8ab[gi % 2]
            s8_v = s8[:].rearrange(
                "p (y k c e) -> p y k c e", y=GROUP, k=KK, c=C, e=2)
            nc.vector.tensor_tensor(
                out=s8_v, in0=g_v[:, :, :, :, 0:2],
                in1=g_v[:, :, :, :, 2:4], op=OP.add)

            st = st_pool.tile([128, GROUP * KTILES * 128], BF16, tag="st")
            st_v = st[:].rearrange(
                "p (m c) -> p m c", m=GROUP * KTILES, c=128)
            nc.scalar.dma_start_transpose(out=st_v, in_=s8[:])
            st_y = st[:].rearrange(
                "p (y j c) -> p y j c", y=GROUP, j=KTILES, c=128)
            ps2 = psum2_pool.tile([O, GROUP * W], F32, tag="ps2")
            for j in range(KTILES):
                nc.tensor.matmul(
                    out=ps2[:], lhsT=w2t_v[:, j, :], rhs=st_y[:, :, j, :],
                    start=(j == 0), stop=(j == KTILES - 1),
                )
            ob = o_pool.tile([O, GROUP * W], F32, tag="ob")
            nc.scalar.activation(
                out=ob[:], in_=ps2[:], func=AF.Identity, bias=b2t[:])
            nc.scalar.dma_start(
                out=out[:, gi * GROUP * W:(gi + 1) * GROUP * W], in_=ob[:])

        # ---------------- sections ----------------
        row0 = 0
        gi0 = 0
        for si, srows in enumerate(secs):
            ysl = slice(row0, row0 + srows)
            msl = slice(row0 * KK, (row0 + srows) * KK)
            mlen = srows * KK

            # offset conv: per 4-row chunk, 3 tap-pair matmuls
            # (contraction 128 = taps (dy,0)+(dy,1)) + 3 singles (dy,2)
            oct_ = oc_pool.tile([32, max(secs) * W], BF16, tag="ocq")
            ocq = oct_[:, 0:srows * W]
            if si < 2:  # oc buffers only ever written on rows 0:18; zero once
                nc.scalar.memzero(oct_[:])
            for cc in range(srows // 4):
                r0 = row0 + cc * 4
                ps = psum_pool.tile([18, 512], F32, tag="ps1")
                for dy in range(3):
                    nc.tensor.matmul(
                        out=ps[:], lhsT=wofp_v[:, dy, :],
                        rhs=xb_v[:, r0 + dy:r0 + dy + 4, 0:W],
                        start=(dy == 0), stop=False,
                    )
                for dy in range(3):
                    nc.tensor.matmul(
                        out=ps[:], lhsT=wof_v[:, 3 * dy + 2, :],
                        rhs=xb_v[0:C, r0 + dy:r0 + dy + 4, 2:2 + W],
                        start=False, stop=(dy == 2),
                    )
                nc.scalar.activation(
                    out=ocq[:18, cc * 512:(cc + 1) * 512], in_=ps[:],
                    func=AF.Identity, bias=bof[:],
                )

            # offsets to pixel-major
            op3q = op3[:, ysl, :]
            nc.sync.dma_start_transpose(out=op3q, in_=ocq)

            # ---- wf/idx prep: contention-free ops only (TT / STT) ----
            # prep scratch tiles are section-sized; lsl is the local slice
            s = pp("s")
            f0 = pp("f0")
            lt = pp("lt")
            lsl = slice(0, mlen)
            s3 = s[:, lsl].rearrange("p (y k) -> p y k", y=srows, k=KK)
            for axis in (0, 1):  # 0: y, 1: x
                grid = cy if axis == 0 else cxf
                grid3 = grid[:].rearrange("p (y k) -> p y k", y=nrows, k=KK)
                nc.vector.tensor_tensor(
                    out=s3, in0=op3q[:, :, axis * 9:axis * 9 + 9],
                    in1=grid3[:, ysl], op=OP.add)
                # round to nearest via +-1.5*2^23 (stage0 scalar add rounds,
                # stage1 tensor subtract undoes; no constant folding across
                # ALU stages), then floor = round - (s < round)
                nc.vector.scalar_tensor_tensor(
                    out=f0[:, lsl], in0=s[:, lsl], scalar=BIG,
                    in1=cvb(0, mlen), op0=OP.add, op1=OP.subtract)
                nc.vector.tensor_tensor(
                    out=lt[:, lsl], in0=s[:, lsl], in1=f0[:, lsl], op=OP.is_lt)
                nc.vector.tensor_tensor(
                    out=f0[:, lsl], in0=f0[:, lsl], in1=lt[:, lsl],
                    op=OP.subtract)
                # fractions
                w1 = pp(f"w1_{axis}", BF16)
                nc.vector.tensor_tensor(
                    out=w1[:, lsl], in0=s[:, lsl], in1=f0[:, lsl],
                    op=OP.subtract)
                w0 = pp(f"w0_{axis}", BF16)
                nc.vector.scalar_tensor_tensor(
                    out=w0[:, lsl], in0=w1[:, lsl], scalar=-1.0,
                    in1=one_b[:].to_broadcast((128, mlen)),
                    op0=OP.mult, op1=OP.add)
                # clamp for index (clamped coords land in the zero pad);
                # x additionally folds in the +266 table offset
                c0 = pp(f"c0_{axis}")
                if axis == 0:
                    nc.vector.scalar_tensor_tensor(
                        out=c0[:, lsl], in0=f0[:, lsl], scalar=-2.0,
                        in1=cvb(1, mlen), op0=OP.max, op1=OP.min)
                else:
                    nc.vector.scalar_tensor_tensor(
                        out=c0[:, lsl], in0=f0[:, lsl],
                        scalar=float(PADT * PW + PADT),
                        in1=cvb(2, mlen), op0=OP.add, op1=OP.max)
                    nc.vector.tensor_tensor(
                        out=c0[:, lsl], in0=c0[:, lsl], in1=cvb(3, mlen),
                        op=OP.min)

            y_w0, y_w1 = pp("w0_0", BF16), pp("w1_0", BF16)
            x_w0, x_w1 = pp("w0_1", BF16), pp("w1_1", BF16)

            # wf[...,(i,j)] = wy_i * wx_j   (k-outer, corner-inner layout)
            for i, wy in enumerate((y_w0, y_w1)):
                for j, wx in enumerate((x_w0, x_w1)):
                    dst = wf_v[:, ysl, :, 2 * i + j].rearrange(
                        "p y k -> p (y k)")
                    nc.vector.tensor_tensor(
                        out=dst, in0=wy[:, lsl], in1=wx[:, lsl], op=OP.mult)

            # idx16 = yc*132 + (xc + 266), fused with the f32->i16 cast
            nc.vector.scalar_tensor_tensor(
                out=idx16[:, msl], in0=pp("c0_0")[:, lsl], scalar=float(PW),
                in1=pp("c0_1")[:, lsl], op0=OP.mult, op1=OP.add)

            # wrapped-16 idx layout for dma_gather: queue q's cpu pair reads
            # partitions 32q..32q+31, so each group's shuffles write ONLY its
            # queue's block (wrapped[32q+16h+p16, m*8+phi] = idx16[p16+16phi,
            # m]). No replication copies/DMAs: every gather's dependency set
            # stays DVE-only, which Tile expresses as one exact engine-tick
            # wait (multi-lane sets get merged into much later events).
            wrt = wr_pool.tile([128, max(secs) * KK * 8], I16, tag="wrapped")
            wrapped = wrt[:, 0:mlen * 8]
            wr_v = wrapped.rearrange("p (m e) -> p m e", m=mlen, e=8)
            GM = GROUP * KK
            for gg in range(srows // GROUP):
                q = 32 * ((gi0 + gg) % 4)
                msl_g = slice((row0 + gg * GROUP) * KK,
                              (row0 + (gg + 1) * GROUP) * KK)
                for qj in range(4):
                    for qh in range(2):
                        mask = [16 * qh + (p % 16) for p in range(32)]
                        nc.vector.stream_shuffle(
                            out=wr_v[q:q + 32, gg * GM:(gg + 1) * GM,
                                     2 * qj + qh],
                            in_=idx16[32 * qj:32 * (qj + 1), msl_g], mask=mask)

            # gathers for this section (queues round-robin)
            for gg in range(srows // GROUP):
                gi = gi0 + gg
                g = g_pool.tile([128, GROUP * KK * EROW], BF16, tag="g")
                gtiles[gi] = g
                g_m = g[:].rearrange("p (m e) -> p m e", m=GROUP * KK, e=EROW)
                nidx_g = 128 * GROUP * KK
                nc.gpsimd.dma_gather(
                    out_ap=g_m, in_ap=table[:],
                    idxs_ap=wrapped[:, gg * GROUP * KK * 8:
                                    (gg + 1) * GROUP * KK * 8],
                    num_idxs=nidx_g, num_idxs_reg=nidx_g, elem_size=EROW,
                    single_packet=False, queue_num=gi % 4)

            gi0 += srows // GROUP
            row0 += srows

        # all consumes after the full gather stream: keeps every gather's
        # DVE-tick wait below the consume ticks (no false serialization)
        for gi in range(nrows // GROUP):
            emit_consume(gi)

        ctx.close()
    nc.compile()
    return nc


# ---------------- host side ----------------

def make_constants():
    """Per-core constant tensors (identical across cores except cgridy)."""
    ky, kx = np.meshgrid(np.arange(3), np.arange(3), indexing="ij")
    ky = ky.reshape(KK).astype(np.float32)
    kx = kx.reshape(KK).astype(np.float32)
    return ky, kx


def host_prepare(x, off_w, off_b, weight, bias, nrows=ROWS):
    """Build per-core input maps."""
    B = x.shape[0]
    x = np.asarray(x, np.float32)
    # padded image for offset conv: [B, C, H+2, W+3]; xband2 stacks the
    # (dx=0) view on channels 0-63 and the (dx=1)-shifted view on 64-127.
    xpad = np.pad(x, ((0, 0), (0, 0), (1, 1), (1, 2))).astype(ml_dtypes.bfloat16)
    # patch table per image: padded-by-2 channels-last
    xp2 = np.pad(x, ((0, 0), (0, 0), (PADT, PADT + 1), (PADT, PADT + 1)))
    # [B, PW+1, PW+1, C] channels-last
    xcl = xp2.transpose(0, 2, 3, 1)
    # table[q=(yp*PW+xp)] = interleave over c: [x(y,x,c), x(y,x+1,c), x(y+1,x,c), x(y+1,x+1,c)]
    tables = []
    for b in range(B):
        t = np.empty((PW, PW, C, 4), np.float32)
        t[:, :, :, 0] = xcl[b, :PW, :PW]
        t[:, :, :, 1] = xcl[b, :PW, 1:PW + 1]
        t[:, :, :, 2] = xcl[b, 1:PW + 1, :PW]
        t[:, :, :, 3] = xcl[b, 1:PW + 1, 1:PW + 1]
        tables.append(t.reshape(NTAB, EROW).astype(ml_dtypes.bfloat16))

    # offset conv weights: channel perm [dy taps 0..8, dx taps 0..8]
    perm = [2 * k for k in range(KK)] + [2 * k + 1 for k in range(KK)]
    w_off_p = np.asarray(off_w, np.float32)[perm]          # [18, C, 3, 3]
    # lhsT per tap: tap k = dy*3+dx -> [C, 18]
    w_off_t = np.empty((KK, C, 18), np.float32)
    for k in range(KK):
        dy, dx = k // 3, k % 3
        w_off_t[k] = w_off_p[:, :, dy, dx].T               # [C, 18]
    # tap-pair lhsT: pair (dy,0) on channels 0-63 with (dy,1) on 64-127
    w_off_pair = np.empty((3, 128, 18), np.float32)
    for dy in range(3):
        w_off_pair[dy, :C] = w_off_t[3 * dy + 0]
        w_off_pair[dy, C:] = w_off_t[3 * dy + 1]
    w_off_t = w_off_t.astype(ml_dtypes.bfloat16)
    w_off_pair = w_off_pair.astype(ml_dtypes.bfloat16)
    b_off_p = np.asarray(off_b, np.float32)[perm].reshape(18, 1)

    # main weights: W2[(k,c,xe), o] = weight[o, c, k] duplicated per
    # x-corner pair (1152 = 9*128 rows exactly)
    wgt = np.asarray(weight, np.float32).reshape(O, C, KK)
    kc = wgt.transpose(2, 1, 0).reshape(KK * C, O)          # [(k,c), O]
    w2f = np.repeat(kc, 2, axis=0)                          # [(k,c,xe), O]
    w2f = w2f.reshape(KTILES, 128, O).astype(ml_dtypes.bfloat16)
    b2f = np.asarray(bias, np.float32).reshape(O, 1)

    ky, kx = make_constants()
    cvals = np.array([[BIG, 128.0, float(PADT * PW + PADT - 2),
                       float(PADT * PW + PADT + 128)]], np.float32)
    cone = np.ones((1, 1), ml_dtypes.bfloat16)

    in_maps = []
    for core in range(8):
        b, hh = core // 2, core % 2
        y0 = hh * 64
        rows = np.arange(y0, y0 + nrows, dtype=np.float32)
        cgy = (rows[:, None] + ky[None, :] - 1.0).reshape(1, nrows * KK)
        # x grid with the per-pixel iota folded in: cxfull[px, (y,k)] =
        # px + kx - 1
        px = np.arange(128, dtype=np.float32)
        cgx = np.broadcast_to(kx[None, :] - 1.0, (nrows, KK)).reshape(nrows * KK)
        cxfull = (px[:, None] + cgx[None, :]).astype(ml_dtypes.bfloat16)
        band = xpad[b, :, y0:y0 + nrows + 2, :]             # [C, nrows+2, XBW]
        xband2 = np.concatenate([band, np.roll(band, -1, axis=2)], axis=0)
        in_maps.append({
            "xband2": np.ascontiguousarray(xband2),
            "table": tables[b],
            "w_off": w_off_t,
            "w_offp": w_off_pair,
            "b_off": b_off_p,
            "w2": w2f,
            "b2": b2f,
            "cgridy": cgy.astype(ml_dtypes.bfloat16),
            "cxfull": np.ascontiguousarray(cxfull),
            "cvals": cvals,
            "cone": cone,
        })
    return in_maps


def assemble(outs, nrows=ROWS):
    """outs: list of 8 dicts with 'out' [O, nrows*W] -> [4, O, H, W]"""
    full = np.zeros((4, O, H, W), np.float32)
    for core, om in enumerate(outs):
        b, hh = core // 2, core % 2
        full[b, :, hh * 64:hh * 64 + nrows] = om["out"].reshape(O, nrows, W)
    return full


_CACHE = {}


def kernel(x, off_w, off_b, weight, bias):
    if "nc" not in _CACHE:
        _CACHE["nc"] = build_program()
    nc = _CACHE["nc"]
    in_maps = host_prepare(x, off_w, off_b, weight, bias)
    from concourse.bass_utils import run_bass_kernel_spmd
    res = run_bass_kernel_spmd(nc, in_maps, core_ids=list(range(8)))
    return assemble(res.results)


# revision 16
# speedup vs baseline: 1.2747x; 1.2216x over previous
"""Trainium2 Bass kernel for DeformableConvBlock (B=4, C=64, H=W=128, K=3).

Self-contained: builds an SPMD Bass/Tile program for 8 NeuronCores.
Core c handles image c//2, output-row half c%2 (data-parallel over
batch x row-halves).

v3 pipeline. Key constraint discovered on HW: the GpSimd Q7 SWDGE
descriptor generation (dma_gather) shares an SBUF port pair with the
DVE's 2-port perf modes, so any tensor_scalar/tensor_copy/memset issued
while a gather is generating blocks until that generation finishes
(~36us). The steady-state therefore uses ONLY contention-free DVE ops
(tensor_tensor / scalar_tensor_tensor / stream_shuffle); index
replication runs as SBUF->SBUF HWDGE DMAs, and the s8 pad columns live
in two persistent buffers zeroed once up front.

Front end per row-section ([8,8,16,16,16]): offset-conv (PE) ->
DMA-transpose to pixel-major -> fused bilinear weight/index prep (DVE)
-> wrapped-index stream_shuffles + replicate DMAs -> dma_gathers
(4608 idx x 512B each, queues round-robin; 4 queues generate
concurrently at ~7.9ns/idx -> 146us floor for 16 gathers).
Bilinear consumes (DVE mult + 2 adds), transpose, PE contraction, bias,
out-DMA are interleaved a few sections behind the gather stream.

kernel(**inputs) takes the full unsharded numpy inputs and returns the
full [4, 64, 128, 128] float32 output.
"""
from contextlib import ExitStack

import numpy as np
import ml_dtypes

import concourse.bacc as bacc
import concourse.bass as bass
import concourse.mybir as mybir
import concourse.tile as tile
from concourse.tile import TileContext
from concourse.vector_clock import ScopedClock, VectorClock

F32 = mybir.dt.float32
BF16 = mybir.dt.bfloat16
I32 = mybir.dt.int32
I16 = mybir.dt.int16
AF = mybir.ActivationFunctionType
OP = mybir.AluOpType

H = W = 128
C = 64
O = 64
KK = 9
ROWS = 64            # output rows per core
PADT = 2             # table padding on each side
PW = W + 2 * PADT    # 132
NTAB = PW * PW       # 17424 table rows
EROW = 4 * C         # 256 elems per table row
KTILES = 9           # 1152 = 9*128 contraction tiles: (k, c, x-corner pair)
GROUP = 2            # rows per gather group = rows per matmul batch
XBW = W + 3          # xband2 column count
BIG = 12582912.0     # 1.5 * 2^23 fp32 rounding offset
SECS = [4, 12, 16, 16, 16]


class TileContextSplitDrain(TileContext):
    """Stock epilogue emits one Drain with one wait per outstanding proc;
    this walrus rejects >1 sync wait per instruction, so emit one Drain
    per proc instead."""

    def _drain_and_barrier(self, tick_clock, wait_clock):
        gc = tick_clock.global_clock
        nprocs = len(gc)
        emitted = False
        for p in range(nprocs):
            t = gc[p]
            if t <= 0:
                continue
            vec = [0] * nprocs
            vec[p] = t
            drain_inst = self.nc.sync.drain()
            wait_clock.add_sem_waits(
                drain_inst.ins, ScopedClock({None: VectorClock(vec)})
            )
            si = drain_inst.ins.sync_info
            assert si is None or len(si.on_wait) <= 1
            emitted = True
        if not emitted:
            self.nc.sync.drain()
        self.nc.all_engine_barrier()
        assert self.sems is not None
        popped = self.nc._tile_sem_poison_stack.pop()
        assert popped is self._sem_poison
        self.nc.clear_and_free_semaphores(list(self.sems.allocated().values()))
        self.nc.all_engine_barrier()


def build_program(nrows=ROWS, g_bufs=13):
    npx = nrows * W
    secs = SECS if nrows >= 64 else [nrows]

    nc = bacc.Bacc(num_swdge_queues=4)
    # xband2: rows 0-63 channel c of padded x (cols 0..XBW-1), rows 64-127
    # the same shifted one column left (cols 1..XBW).
    xband2 = nc.dram_tensor("xband2", [128, nrows + 2, XBW], BF16, kind="ExternalInput")
    table = nc.dram_tensor("table", [NTAB, EROW], BF16, kind="ExternalInput")
    w_off = nc.dram_tensor("w_off", [KK, C, 18], BF16, kind="ExternalInput")
    w_offp = nc.dram_tensor("w_offp", [3, 128, 18], BF16, kind="ExternalInput")
    b_off = nc.dram_tensor("b_off", [18, 1], F32, kind="ExternalInput")
    w2 = nc.dram_tensor("w2", [KTILES, 128, O], BF16, kind="ExternalInput")
    b2 = nc.dram_tensor("b2", [O, 1], F32, kind="ExternalInput")
    cgridy = nc.dram_tensor("cgridy", [1, nrows * KK], BF16, kind="ExternalInput")
    cxfull = nc.dram_tensor("cxfull", [128, nrows * KK], BF16, kind="ExternalInput")
    cvals = nc.dram_tensor("cvals", [1, 4], F32, kind="ExternalInput")
    cone = nc.dram_tensor("cone", [1, 1], BF16, kind="ExternalInput")
    out = nc.dram_tensor("out", [O, npx], F32, kind="ExternalOutput")

    ctx = ExitStack()
    with TileContextSplitDrain(nc) as tc:
        # wrapped-idx tiles only fill the 32-partition block the gather's
        # queue actually reads; other blocks keep stale (in-bounds) data
        # that the race detector would flag
        tc.race_detector_enabled = False
        const_pool = ctx.enter_context(tc.tile_pool(name="const", bufs=1))
        big_pool = ctx.enter_context(tc.tile_pool(name="big", bufs=1))
        prep_pool = ctx.enter_context(tc.tile_pool(name="prep", bufs=1))
        oc_pool = ctx.enter_context(tc.tile_pool(name="oc", bufs=2))
        wr_pool = ctx.enter_context(tc.tile_pool(name="wr", bufs=5))
        g_pool = ctx.enter_context(tc.tile_pool(name="g", bufs=g_bufs))
        st_pool = ctx.enter_context(tc.tile_pool(name="st", bufs=2))
        o_pool = ctx.enter_context(tc.tile_pool(name="o", bufs=2))
        psum_pool = ctx.enter_context(tc.tile_pool(name="ps", bufs=2, space="PSUM"))
        psum2_pool = ctx.enter_context(tc.tile_pool(name="ps2", bufs=2, space="PSUM"))

        # ---- constant loads ----
        xb = const_pool.tile([128, (nrows + 2) * XBW], BF16)
        xb_v = xb[:].rearrange("c (h w) -> c h w", h=nrows + 2, w=XBW)
        # split band load so the first offset-conv section starts early
        xsplit = min(secs[0] + 6, nrows + 2)
        nc.sync.dma_start(out=xb_v[:, 0:xsplit], in_=xband2[:, 0:xsplit])
        nc.sync.dma_start(out=xb_v[:, xsplit:], in_=xband2[:, xsplit:])

        wof = const_pool.tile([C, KK * 18], BF16)
        wof_v = wof[:].rearrange("c (k e) -> c k e", k=KK, e=18)
        nc.sync.dma_start(out=wof_v, in_=w_off[:].rearrange("k c e -> c k e"))

        wofp = const_pool.tile([128, 3 * 18], BF16)
        wofp_v = wofp[:].rearrange("c (k e) -> c k e", k=3, e=18)
        nc.sync.dma_start(out=wofp_v, in_=w_offp[:].rearrange("k c e -> c k e"))

        bof = const_pool.tile([18, 1], F32)
        nc.sync.dma_start(out=bof[:], in_=b_off[:])

        w2t = const_pool.tile([128, KTILES * O], BF16)
        w2t_v = w2t[:].rearrange("p (j e) -> p j e", j=KTILES, e=O)
        nc.sync.dma_start(out=w2t_v, in_=w2[:].rearrange("j p e -> p j e"))

        b2t = const_pool.tile([O, 1], F32)
        nc.sync.dma_start(out=b2t[:], in_=b2[:])

        cy = const_pool.tile([128, nrows * KK], BF16)
        nc.sync.dma_start(out=cy[:], in_=cgridy[0:1, :].to_broadcast((128, nrows * KK)))
        cxf = const_pool.tile([128, nrows * KK], BF16)
        nc.sync.dma_start(out=cxf[:], in_=cxfull[:])
        cv = const_pool.tile([128, 4], F32)
        nc.sync.dma_start(out=cv[:], in_=cvals[0:1, :].to_broadcast((128, 4)))
        one_b = const_pool.tile([128, 1], BF16)
        nc.sync.dma_start(out=one_b[:], in_=cone[0:1, :].to_broadcast((128, 1)))

        def cvb(i, n):  # broadcast view of constant column i
            return cv[:, i:i + 1].to_broadcast((128, n))

        # ---- persistent whole-half tiles ----
        op = big_pool.tile([128, nrows * 32], BF16)
        op3 = op[:].rearrange("p (y e) -> p y e", y=nrows, e=32)
        wf = big_pool.tile([128, nrows * 36], BF16)
        wf_v = wf[:].rearrange("p (y k c) -> p y k c", y=nrows, k=KK, c=4)
        idx16 = big_pool.tile([128, nrows * KK], I16)
        # two persistent s8 buffers (1152 = 9*64*2 used exactly, no padding)
        s8ab = [
            big_pool.tile([128, GROUP * KTILES * 128], BF16,
                          tag=f"s8_{sbi}", name=f"s8_{sbi}")
            for sbi in range(2)
        ]

        ptiles = {}

        max_mlen = max(secs) * KK

        def pp(tag, dt=F32):
            if tag not in ptiles:
                ptiles[tag] = prep_pool.tile(
                    [128, max_mlen], dt, tag=tag, name=f"prep_{tag}")
            return ptiles[tag]

        gtiles = [None] * (nrows // GROUP)

        def emit_consume(gi):
            y0 = gi * GROUP
            g = gtiles[gi]
            g_v = g[:].rearrange(
                "p (y k c e) -> p y k c e", y=GROUP, k=KK, c=C, e=4)
            wfb = wf_v[:, y0:y0 + GROUP, :, None, :].to_broadcast(
                (128, GROUP, KK, C, 4))
            nc.vector.tensor_tensor(out=g_v, in0=g_v, in1=wfb, op=OP.mult)
            # y-corner pairwise add, compacted into s8: s8[y,k,c,xe] =
            # g[...,xe] + g[...,xe+2]; the x-corner pair stays in the PE
            # contraction (weights duplicated per pair)
            s8 = s8ab[gi % 2]
            s8_v = s8[:].rearrange(
                "p (y k c e) -> p y k c e", y=GROUP, k=KK, c=C, e=2)
            nc.vector.tensor_tensor(
                out=s8_v, in0=g_v[:, :, :, :, 0:2],
                in1=g_v[:, :, :, :, 2:4], op=OP.add)

            st = st_pool.tile([128, GROUP * KTILES * 128], BF16, tag="st")
            st_v = st[:].rearrange(
                "p (m c) -> p m c", m=GROUP * KTILES, c=128)
            nc.scalar.dma_start_transpose(out=st_v, in_=s8[:])
            st_y = st[:].rearrange(
                "p (y j c) -> p y j c", y=GROUP, j=KTILES, c=128)
            ps2 = psum2_pool.tile([O, GROUP * W], F32, tag="ps2")
            for j in range(KTILES):
                nc.tensor.matmul(
                    out=ps2[:], lhsT=w2t_v[:, j, :], rhs=st_y[:, :, j, :],
                    start=(j == 0), stop=(j == KTILES - 1),
                )
            ob = o_pool.tile([O, GROUP * W], F32, tag="ob")
            nc.scalar.activation(
                out=ob[:], in_=ps2[:], func=AF.Identity, bias=b2t[:])
            nc.scalar.dma_start(
                out=out[:, gi * GROUP * W:(gi + 1) * GROUP * W], in_=ob[:])

        # ---------------- sections ----------------
        row0 = 0
        gi0 = 0
        for si, srows in enumerate(secs):
            ysl = slice(row0, row0 + srows)
            msl = slice(row0 * KK, (row0 + srows) * KK)
            mlen = srows * KK

            # offset conv: per 4-row chunk, 3 tap-pair matmuls
            # (contraction 128 = taps (dy,0)+(dy,1)) + 3 singles (dy,2)
            oct_ = oc_pool.tile([32, max(secs) * W], BF16, tag="ocq")
            ocq = oct_[:, 0:srows * W]
            if si < 2:  # oc buffers only ever written on rows 0:18; zero once
                nc.scalar.memzero(oct_[:])
            for cc in range(srows // 4):
                r0 = row0 + cc * 4
                ps = psum_pool.tile([18, 512], F32, tag="ps1")
                for dy in range(3):
                    nc.tensor.matmul(
                        out=ps[:], lhsT=wofp_v[:, dy, :],
                        rhs=xb_v[:, r0 + dy:r0 + dy + 4, 0:W],
                        start=(dy == 0), stop=False,
                    )
                for dy in range(3):
                    nc.tensor.matmul(
                        out=ps[:], lhsT=wof_v[:, 3 * dy + 2, :],
                        rhs=xb_v[0:C, r0 + dy:r0 + dy + 4, 2:2 + W],
                        start=False, stop=(dy == 2),
                    )
                nc.scalar.activation(
                    out=ocq[:18, cc * 512:(cc + 1) * 512], in_=ps[:],
                    func=AF.Identity, bias=bof[:],
                )

            # offsets to pixel-major
            op3q = op3[:, ysl, :]
            nc.sync.dma_start_transpose(out=op3q, in_=ocq)

            # ---- wf/idx prep: contention-free ops only (TT / STT) ----
            # prep scratch tiles are section-sized; lsl is the local slice
            s = pp("s")
            f0 = pp("f0")
            lt = pp("lt")
            lsl = slice(0, mlen)
            s3 = s[:, lsl].rearrange("p (y k) -> p y k", y=srows, k=KK)
            for axis in (0, 1):  # 0: y, 1: x
                grid = cy if axis == 0 else cxf
                grid3 = grid[:].rearrange("p (y k) -> p y k", y=nrows, k=KK)
                nc.vector.tensor_tensor(
                    out=s3, in0=op3q[:, :, axis * 9:axis * 9 + 9],
                    in1=grid3[:, ysl], op=OP.add)
                # round to nearest via +-1.5*2^23 (stage0 scalar add rounds,
                # stage1 tensor subtract undoes; no constant folding across
                # ALU stages), then floor = round - (s < round)
                nc.vector.scalar_tensor_tensor(
                    out=f0[:, lsl], in0=s[:, lsl], scalar=BIG,
                    in1=cvb(0, mlen), op0=OP.add, op1=OP.subtract)
                nc.vector.tensor_tensor(
                    out=lt[:, lsl], in0=s[:, lsl], in1=f0[:, lsl], op=OP.is_lt)
                nc.vector.tensor_tensor(
                    out=f0[:, lsl], in0=f0[:, lsl], in1=lt[:, lsl],
                    op=OP.subtract)
                # fractions
                w1 = pp(f"w1_{axis}", BF16)
                nc.vector.tensor_tensor(
                    out=w1[:, lsl], in0=s[:, lsl], in1=f0[:, lsl],
                    op=OP.subtract)
                w0 = pp(f"w0_{axis}", BF16)
                nc.vector.scalar_tensor_tensor(
                    out=w0[:, lsl], in0=w1[:, lsl], scalar=-1.0,
                    in1=one_b[:].to_broadcast((128, mlen)),
                    op0=OP.mult, op1=OP.add)
                # clamp for index (clamped coords land in the zero pad);
                # x additionally folds in the +266 table offset
                c0 = pp(f"c0_{axis}")
                if axis == 0:
                    nc.vector.scalar_tensor_tensor(
                        out=c0[:, lsl], in0=f0[:, lsl], scalar=-2.0,
                        in1=cvb(1, mlen), op0=OP.max, op1=OP.min)
                else:
                    nc.vector.scalar_tensor_tensor(
                        out=c0[:, lsl], in0=f0[:, lsl],
                        scalar=float(PADT * PW + PADT),
                        in1=cvb(2, mlen), op0=OP.add, op1=OP.max)
                    nc.vector.tensor_tensor(
                        out=c0[:, lsl], in0=c0[:, lsl], in1=cvb(3, mlen),
                        op=OP.min)

            y_w0, y_w1 = pp("w0_0", BF16), pp("w1_0", BF16)
            x_w0, x_w1 = pp("w0_1", BF16), pp("w1_1", BF16)

            # wf[...,(i,j)] = wy_i * wx_j   (k-outer, corner-inner layout)
            for i, wy in enumerate((y_w0, y_w1)):
                for j, wx in enumerate((x_w0, x_w1)):
                    dst = wf_v[:, ysl, :, 2 * i + j].rearrange(
                        "p y k -> p (y k)")
                    nc.vector.tensor_tensor(
                        out=dst, in0=wy[:, lsl], in1=wx[:, lsl], op=OP.mult)

            # idx16 = yc*132 + (xc + 266), fused with the f32->i16 cast
            nc.vector.scalar_tensor_tensor(
                out=idx16[:, msl], in0=pp("c0_0")[:, lsl], scalar=float(PW),
                in1=pp("c0_1")[:, lsl], op0=OP.mult, op1=OP.add)

            # wrapped-16 idx layout for dma_gather: queue q's cpu pair reads
            # partitions 32q..32q+31, so each group's shuffles write ONLY its
            # queue's block (wrapped[32q+16h+p16, m*8+phi] = idx16[p16+16phi,
            # m]). No replication copies/DMAs: every gather's dependency set
            # stays DVE-only, which Tile expresses as one exact engine-tick
            # wait (multi-lane sets get merged into much later events).
            wrt = wr_pool.tile([128, max(secs) * KK * 8], I16, tag="wrapped")
            wrapped = wrt[:, 0:mlen * 8]
            wr_v = wrapped.rearrange("p (m e) -> p m e", m=mlen, e=8)
            GM = GROUP * KK
            for gg in range(srows // GROUP):
                q = 32 * ((gi0 + gg) % 4)
                msl_g = slice((row0 + gg * GROUP) * KK,
                              (row0 + (gg + 1) * GROUP) * KK)
                for qj in range(4):
                    for qh in range(2):
                        mask = [16 * qh + (p % 16) for p in range(32)]
                        nc.vector.stream_shuffle(
                            out=wr_v[q:q + 32, gg * GM:(gg + 1) * GM,
                                     2 * qj + qh],
                            in_=idx16[32 * qj:32 * (qj + 1), msl_g], mask=mask)

            # gathers for this section (queues round-robin)
            for gg in range(srows // GROUP):
                gi = gi0 + gg
                g = g_pool.tile([128, GROUP * KK * EROW], BF16, tag="g")
                gtiles[gi] = g
                g_m = g[:].rearrange("p (m e) -> p m e", m=GROUP * KK, e=EROW)
                nidx_g = 128 * GROUP * KK
                nc.gpsimd.dma_gather(
                    out_ap=g_m, in_ap=table[:],
                    idxs_ap=wrapped[:, gg * GROUP * KK * 8:
                                    (gg + 1) * GROUP * KK * 8],
                    num_idxs=nidx_g, num_idxs_reg=nidx_g, elem_size=EROW,
                    single_packet=False, queue_num=gi % 4)

            gi0 += srows // GROUP
            row0 += srows

        # all consumes after the full gather stream. tile_wait_until pins
        # them late in every engine's STATIC order: the scheduler's cost
        # model thinks a gather takes ~1.6us (vs ~22us real), so without
        # this it interleaves gather-dependent consumes into the DVE queue,
        # head-blocking it ~25us per consume.
        for gi in range(nrows // GROUP):
            with tc.tile_wait_until(1.0 + 0.001 * gi):
                emit_consume(gi)

        ctx.close()
    nc.compile()
    return nc


# ---------------- host side ----------------

def make_constants():
    """Per-core constant tensors (identical across cores except cgridy)."""
    ky, kx = np.meshgrid(np.arange(3), np.arange(3), indexing="ij")
    ky = ky.reshape(KK).astype(np.float32)
    kx = kx.reshape(KK).astype(np.float32)
    return ky, kx


def host_prepare(x, off_w, off_b, weight, bias, nrows=ROWS):
    """Build per-core input maps."""
    B = x.shape[0]
    x = np.asarray(x, np.float32)
    # padded image for offset conv: [B, C, H+2, W+3]; xband2 stacks the
    # (dx=0) view on channels 0-63 and the (dx=1)-shifted view on 64-127.
    xpad = np.pad(x, ((0, 0), (0, 0), (1, 1), (1, 2))).astype(ml_dtypes.bfloat16)
    # patch table per image: padded-by-2 channels-last
    xp2 = np.pad(x, ((0, 0), (0, 0), (PADT, PADT + 1), (PADT, PADT + 1)))
    # [B, PW+1, PW+1, C] channels-last
    xcl = xp2.transpose(0, 2, 3, 1)
    # table[q=(yp*PW+xp)] = interleave over c: [x(y,x,c), x(y,x+1,c), x(y+1,x,c), x(y+1,x+1,c)]
    tables = []
    for b in range(B):
        t = np.empty((PW, PW, C, 4), np.float32)
        t[:, :, :, 0] = xcl[b, :PW, :PW]
        t[:, :, :, 1] = xcl[b, :PW, 1:PW + 1]
        t[:, :, :, 2] = xcl[b, 1:PW + 1, :PW]
        t[:, :, :, 3] = xcl[b, 1:PW + 1, 1:PW + 1]
        tables.append(t.reshape(NTAB, EROW).astype(ml_dtypes.bfloat16))

    # offset conv weights: channel perm [dy taps 0..8, dx taps 0..8]
    perm = [2 * k for k in range(KK)] + [2 * k + 1 for k in range(KK)]
    w_off_p = np.asarray(off_w, np.float32)[perm]          # [18, C, 3, 3]
    # lhsT per tap: tap k = dy*3+dx -> [C, 18]
    w_off_t = np.empty((KK, C, 18), np.float32)
    for k in range(KK):
        dy, dx = k // 3, k % 3
        w_off_t[k] = w_off_p[:, :, dy, dx].T               # [C, 18]
    # tap-pair lhsT: pair (dy,0) on channels 0-63 with (dy,1) on 64-127
    w_off_pair = np.empty((3, 128, 18), np.float32)
    for dy in range(3):
        w_off_pair[dy, :C] = w_off_t[3 * dy + 0]
        w_off_pair[dy, C:] = w_off_t[3 * dy + 1]
    w_off_t = w_off_t.astype(ml_dtypes.bfloat16)
    w_off_pair = w_off_pair.astype(ml_dtypes.bfloat16)
    b_off_p = np.asarray(off_b, np.float32)[perm].reshape(18, 1)

    # main weights: W2[(k,c,xe), o] = weight[o, c, k] duplicated per
    # x-corner pair (1152 = 9*128 rows exactly)
    wgt = np.asarray(weight, np.float32).reshape(O, C, KK)
    kc = wgt.transpose(2, 1, 0).reshape(KK * C, O)          # [(k,c), O]
    w2f = np.repeat(kc, 2, axis=0)                          # [(k,c,xe), O]
    w2f = w2f.reshape(KTILES, 128, O).astype(ml_dtypes.bfloat16)
    b2f = np.asarray(bias, np.float32).reshape(O, 1)

    ky, kx = make_constants()
    cvals = np.array([[BIG, 128.0, float(PADT * PW + PADT - 2),
                       float(PADT * PW + PADT + 128)]], np.float32)
    cone = np.ones((1, 1), ml_dtypes.bfloat16)

    in_maps = []
    for core in range(8):
        b, hh = core // 2, core % 2
        y0 = hh * 64
        rows = np.arange(y0, y0 + nrows, dtype=np.float32)
        cgy = (rows[:, None] + ky[None, :] - 1.0).reshape(1, nrows * KK)
        # x grid with the per-pixel iota folded in: cxfull[px, (y,k)] =
        # px + kx - 1
        px = np.arange(128, dtype=np.float32)
        cgx = np.broadcast_to(kx[None, :] - 1.0, (nrows, KK)).reshape(nrows * KK)
        cxfull = (px[:, None] + cgx[None, :]).astype(ml_dtypes.bfloat16)
        band = xpad[b, :, y0:y0 + nrows + 2, :]             # [C, nrows+2, XBW]
        xband2 = np.concatenate([band, np.roll(band, -1, axis=2)], axis=0)
        in_maps.append({
            "xband2": np.ascontiguousarray(xband2),
            "table": tables[b],
            "w_off": w_off_t,
            "w_offp": w_off_pair,
            "b_off": b_off_p,
            "w2": w2f,
            "b2": b2f,
            "cgridy": cgy.astype(ml_dtypes.bfloat16),
            "cxfull": np.ascontiguousarray(cxfull),
            "cvals": cvals,
            "cone": cone,
        })
    return in_maps


def assemble(outs, nrows=ROWS):
    """outs: list of 8 dicts with 'out' [O, nrows*W] -> [4, O, H, W]"""
    full = np.zeros((4, O, H, W), np.float32)
    for core, om in enumerate(outs):
        b, hh = core // 2, core % 2
        full[b, :, hh * 64:hh * 64 + nrows] = om["out"].reshape(O, nrows, W)
    return full


_CACHE = {}


def kernel(x, off_w, off_b, weight, bias):
    if "nc" not in _CACHE:
        _CACHE["nc"] = build_program()
    nc = _CACHE["nc"]
    in_maps = host_prepare(x, off_w, off_b, weight, bias)
    from concourse.bass_utils import run_bass_kernel_spmd
    res = run_bass_kernel_spmd(nc, in_maps, core_ids=list(range(8)))
    return assemble(res.results)
